# revision 15
# baseline (speedup 1.0000x reference)
"""Trainium2 kernel for nn_KernelizedAttention_14869176779022.

Math note: the reference computes
    out = (s * v) / s        with s = <phi_q, phi_k> > 0  (sums of exps)
so out == v == x @ Wv.T + bv exactly (up to one multiply/divide rounding).
The kernel therefore only computes the Wv linear layer.

Sharding: data-parallel over the 8192 (B*S) positions - 1024 rows per core.
Wv (pre-transposed) and bv are replicated. x is pre-swizzled on the host into
the exact SBUF layout the TensorEngine wants (contraction dim on partitions).

v2 restructure (v1 measured 58us, PE floor is ~27.5us):
  - Loads via HWDGE (nc.sync / nc.scalar) instead of SWDGE: no ~9us Q7
    descriptor-generation latency before the first byte moves.
  - DMA chunks ordered in exact consumption order; matmuls start at ~1.5us
    and chase the stream instead of waiting for the full 4MB load.
  - Two n-half supersteps (m-outer, k-inner) so only half of Wv (1MB) is
    needed early; per-m staggered PSUM drains and stores.
  - Output stored as bf16 (halves store bytes; host upcasts). Adds ~1e-3
    fro error on top of the ~4e-4 bf16-matmul error; tolerance is 2e-2.
"""

import sys

if "/opt/trn_rl_repo" not in sys.path:
    sys.path.insert(0, "/opt/trn_rl_repo")

import numpy as np

B, S, E = 2, 4096, 1024
N_CORES = 8
ROWS = B * S            # 8192
R = ROWS // N_CORES     # 1024 rows per core
P = 128                 # partitions
KT = E // P             # 8 contraction tiles
MT = R // P             # 8 row tiles per core
NH = 2                  # n-half supersteps (512 output cols each)
NSZ = E // NH           # 512 = one PSUM bank (fp32)

_NC_CACHE = {}


def _build_nc(**bass_kwargs):
    import concourse.bass as bass
    import concourse.mybir as mybir
    from concourse import bacc
    from concourse.tile import TileContext

    f32 = mybir.dt.float32
    bf16 = mybir.dt.bfloat16
    nc = bacc.Bacc(None, target_bir_lowering=False, **bass_kwargs)

    # xb[p, (m*KT + k)*P + mm] = x_shard[m*P + mm, k*P + p]   (bf16, host-packed)
    xb = nc.dram_tensor("xb", [P, MT * KT * P], bf16, kind="ExternalInput")
    # wv[p, (h*KT + k)*NSZ + c] = Wv[h*NSZ + c, k*P + p]       (bf16, host-packed)
    wv = nc.dram_tensor("wv", [P, NH * KT * NSZ], bf16, kind="ExternalInput")
    # bias pre-broadcast on the host to all 128 partitions (bf16)
    bvb = nc.dram_tensor("bvb", [P, E], bf16, kind="ExternalInput")
    out = nc.dram_tensor("out", [R, E], bf16, kind="ExternalOutput")

    with TileContext(nc) as tc:
        with (
            tc.tile_pool(name="consts", bufs=1) as consts,
            tc.tile_pool(name="xpool", bufs=1) as xpool,
            tc.tile_pool(name="wpool", bufs=1) as wpool,
            tc.tile_pool(name="opool", bufs=MT) as opool,
            tc.tile_pool(name="ppool", bufs=8, space="PSUM") as ppool,
        ):
            # PE warm-up: dummy matmuls on a scratch tile fill the gap between
            # preamble-end (~7.6us) and first input chunk (~9us) so the PE is
            # continuously busy and the HAM clock-gate releases (1.2 -> 2.4
            # GHz) as early as possible. Never read; costs one PSUM bank.
            dum_sb = consts.tile([P, NSZ], bf16, tag="dum")
            nc.gpsimd.memset(dum_sb, 0.0)
            dum_ps = ppool.tile([P, NSZ], f32, tag="ps")
            for _ in range(8):
                nc.tensor.matmul(
                    dum_ps, dum_sb[:, :P], dum_sb, start=True, stop=True
                )

            bias_sb = consts.tile([P, E], bf16, tag="bias")
            wv_sb = wpool.tile([P, NH * KT * NSZ], bf16, tag="wv")
            x_sb = xpool.tile([P, MT * KT * P], bf16, tag="x")

            # Queue throughput is ~150 GB/s each and scales with per-partition
            # line size (packet-granular round-robin), so every chunk is
            # >=256KB with >=2KB contiguous lines, balanced over 3 queues in
            # global need-order:
            #   SP ring:  x m0m1, wv h0 k2k3, k6k7, x m6m7
            #   ACT ring: wv h0 k0k1, k4k5, wv h1 (2 x 512KB)
            #   PL SWDGE: bias, x m2m3, x m4m5
            # Stores (16 x 128KB) alternate SP/ACT as each half-drain lands.
            xm = KT * P             # one m-tile of x = 256KB
            wk = NSZ                # one k-tile of one n-half = 128KB

            # ACT ring: wv only (8 x 256KB k-pair chunks, h0 then h1).
            # SP ring:  x only (4 x 512KB m-pair chunks).
            # PL SWDGE: bias, then all 16 output stores.
            xm = KT * P             # one m-tile of x = 256KB
            wk = NSZ                # one k-tile of one n-half = 128KB

            nc.gpsimd.dma_start(out=bias_sb, in_=bvb[:, :])
            for c in range(8):
                nc.scalar.dma_start(
                    out=wv_sb[:, 2 * c * wk : 2 * (c + 1) * wk],
                    in_=wv[:, 2 * c * wk : 2 * (c + 1) * wk],
                )
            for c in range(4):
                nc.sync.dma_start(
                    out=x_sb[:, 2 * c * xm : 2 * (c + 1) * xm],
                    in_=xb[:, 2 * c * xm : 2 * (c + 1) * xm],
                )

            def drain_and_store(h, m, ps):
                om = opool.tile([P, NSZ], bf16, name=f"om{h}_{m}", tag="om")
                nc.vector.tensor_add(
                    out=om,
                    in0=ps,
                    in1=bias_sb[:, h * NSZ : (h + 1) * NSZ],
                )
                dst = bass.AP(
                    tensor=out.tensor if hasattr(out, "tensor") else out,
                    offset=m * P * E + h * NSZ,
                    ap=[[E, P], [1, NSZ]],
                )
                nc.gpsimd.dma_start(out=dst, in_=om)

            # A-pass (h=0): m-pair blocks, k-outer inside each pair, so the
            # PE chases the k-ordered wv h0 stream as chunks arrive.
            for pair in range(MT // 2):
                ma, mb = 2 * pair, 2 * pair + 1
                psa = ppool.tile([P, NSZ], f32, name=f"psa{ma}", tag="ps")
                psb = ppool.tile([P, NSZ], f32, name=f"psa{mb}", tag="ps")
                for k in range(KT):
                    for m, ps in ((ma, psa), (mb, psb)):
                        nc.tensor.matmul(
                            ps,
                            x_sb[:, (m * KT + k) * P : (m * KT + k + 1) * P],
                            wv_sb[:, k * NSZ : (k + 1) * NSZ],
                            start=(k == 0),
                            stop=(k == KT - 1),
                        )
                drain_and_store(0, ma, psa)
                drain_and_store(0, mb, psb)

            # B-pass (h=1): m-outer, wv h1 fully resident by now.
            for m in range(MT):
                ps = ppool.tile([P, NSZ], f32, name=f"psb{m}", tag="ps")
                for k in range(KT):
                    nc.tensor.matmul(
                        ps,
                        x_sb[:, (m * KT + k) * P : (m * KT + k + 1) * P],
                        wv_sb[:, (KT + k) * NSZ : (KT + k + 1) * NSZ],
                        start=(k == 0),
                        stop=(k == KT - 1),
                    )
                drain_and_store(1, m, ps)
    nc.compile()
    return nc


def _get_nc():
    if "nc" not in _NC_CACHE:
        _NC_CACHE["nc"] = _build_nc()
    return _NC_CACHE["nc"]


def _prep_in_maps(x, Wv, bv):
    import ml_dtypes

    bf16 = ml_dtypes.bfloat16
    x = np.ascontiguousarray(np.asarray(x, dtype=np.float32))
    Wv = np.asarray(Wv, dtype=np.float32)
    bv = np.asarray(bv, dtype=np.float32)

    xf = x.reshape(ROWS, E)
    # wvb[p, (h*KT + k)*NSZ + c] = Wv[h*NSZ + c, k*P + p]
    #   [j=(h c), (k p)] -> [p, (h k c)]
    wvp = np.ascontiguousarray(
        Wv.reshape(NH, NSZ, KT, P)
        .transpose(3, 0, 2, 1)
        .reshape(P, NH * KT * NSZ)
        .astype(bf16)
    )
    bv2 = np.ascontiguousarray(
        np.broadcast_to(bv.reshape(1, E), (P, E)).astype(bf16)
    )

    in_maps = []
    for c in range(N_CORES):
        xs = xf[c * R : (c + 1) * R]                    # [R, E]
        # xb[p, (m*KT+k)*P+mm] = xs[m*P+mm, k*P+p]
        xbc = np.ascontiguousarray(
            xs.reshape(MT, P, KT, P)
            .transpose(3, 0, 2, 1)
            .reshape(P, MT * KT * P)
            .astype(bf16)
        )
        in_maps.append({"xb": xbc, "wv": wvp, "bvb": bv2})
    return in_maps


def _install_ntff_hook():
    """This image's antenv lacks axon_hooks; recreate the bridge module so
    run_bass_kernel_spmd(trace=True) can reach the ctypes NTFF profiler."""
    import types

    if "antenv.axon_hooks" in sys.modules:
        return
    try:
        from trn_agent_boot.trn_boot import _ntff_profile_via_ctypes
    except ImportError:
        return
    hook = _ntff_profile_via_ctypes("/opt/axon/libaxon_pjrt.so")
    mod = types.ModuleType("antenv.axon_hooks")
    mod._hook = hook
    mod.get_axon_ntff_profile_hook = lambda: mod._hook
    mod.set_axon_ntff_profile_hook = lambda h: setattr(mod, "_hook", h)
    sys.modules["antenv.axon_hooks"] = mod


def _run(x, Wv, bv, trace=False):
    from concourse.bass_utils import run_bass_kernel_spmd

    if trace:
        _install_ntff_hook()
    nc = _get_nc()
    in_maps = _prep_in_maps(x, Wv, bv)
    res = run_bass_kernel_spmd(
        nc, in_maps, core_ids=list(range(N_CORES)), trace=trace
    )
    out = np.concatenate(
        [np.asarray(res.results[c]["out"]) for c in range(N_CORES)], axis=0
    )
    return out.reshape(B, S, E).astype(np.float32), res


def kernel(x, Wq, bq, Wk, bk, Wv, bv, weights):
    out, _ = _run(x, Wv, bv, trace=False)
    return out


def kernel_traced(x, Wq, bq, Wk, bk, Wv, bv, weights):
    """Like kernel() but with NTFF profiling; returns (out, BassKernelResults)."""
    out, res = _run(x, Wv, bv, trace=True)
    return out, res
